# revision 5
# baseline (speedup 1.0000x reference)
"""Chamfer loss kernel v2 for Trainium2 (Bass/Tile), SPMD over 8 NeuronCores.

Per-core algorithm (core b handles batch b):
  D-tilde[n, m] = <p1[n], p2[m]> - |p2[m]|^2/2   (K=6 fp32r matmul:
      A = [-1,-1,-1, x1,y1,z1] (host data), B = [sx,sy,sz, x2,y2,z2]
      where s* = coord^2/2 computed on-device by one DVE
      scalar_tensor_tensor per half)
  min_m d^2[n] = r1[n] - 2 max_m D-tilde  -- the stationary set's norm is
  applied AFTER the reduction (per-partition), so no r-row reshape DMAs.
  Two symmetric passes make both min directions free-axis reductions;
  per [128, 2048] PSUM tile the ScalarE copies one half to SBUF (fp32 to
  keep absolute precision of the +r1/2-offset values) and one DVE
  tensor_tensor_scan (op0=op1=max) reduces both halves at 2 cols/cycle.
  Tail: r-norm tiles [128,16] from a host-shipped wide view via an exact
  fp32 mask-matmul + PE transpose; d^2 = r - 2*max, clamp, sqrt with
  free-axis accumulation, ones-matmul partition sum, scale.
"""

import numpy as np
from contextlib import ExitStack

import bass_rust
import concourse.bass as bass
import concourse.tile as tile
from concourse import mybir
from concourse.bass_utils import run_bass_kernel_spmd
from concourse.vector_clock import ScopedClock
from concourse.tile import add_dep_helper

F32 = mybir.dt.float32
F32R = mybir.dt.float32r
F16 = mybir.dt.float16
ALU = mybir.AluOpType

B = 8
N = 2048
NB = N // 128
HALF_SQRT2 = 0.7071067811865476


def _split_drain_and_barrier(self, tick_clock, wait_clock):
    """Walrus in this container rejects >1 sync wait per instruction; split
    the kernel-tail drain's waits across single-wait SP nops first."""
    gc = tick_clock.global_clock
    for proc, val in enumerate(gc):
        if val <= 0:
            continue
        v = bass_rust.VectorClock()
        v.require_at_least(proc, val)
        nop = self.nc.sync.nop()
        wait_clock.add_sem_waits(nop.ins, ScopedClock({None: v}))
    self.nc.sync.drain()
    self.nc.all_engine_barrier()
    assert self.sems is not None
    popped = self.nc._tile_sem_poison_stack.pop()
    assert popped is self._sem_poison
    self.nc.clear_and_free_semaphores(list(self.sems.allocated().values()))
    self.nc.all_engine_barrier()


tile.TileContext._drain_and_barrier = _split_drain_and_barrier


def _cap_sync_waits(nc, maxw=1):
    """Hoist excess sync waits onto same-engine NoOps (walrus 1-wait limit)."""
    cnt = 0
    for f in nc.m.functions:
        for blk in f.blocks:
            out = []
            for ins in blk.instructions:
                si = ins.sync_info
                if si is not None and si.on_wait and len(si.on_wait) > maxw:
                    waits = list(si.on_wait)
                    extra, keep = waits[:-maxw], waits[-maxw:]
                    for i in range(0, len(extra), maxw):
                        cnt += 1
                        nop = mybir.InstNoOp(name=f"capw-{cnt}", ins=[], outs=[])
                        nop.engine = ins.engine
                        nop.sync_info = mybir.SyncInfo(
                            on_wait=extra[i : i + maxw], on_update=[]
                        )
                        out.append(nop)
                    ins.sync_info = mybir.SyncInfo(
                        on_wait=keep, on_update=list(si.on_update)
                    )
                out.append(ins)
            blk.instructions[:] = out
    return cnt


def _build_wm():
    # [128, 161] const: cols 0:128 wide view of coords (per-core, filled by
    # make_in_maps), 128:144 the -0.5 r-replication mask, 144:160 eye(16)
    # (at partition rows 0:16 and 32:48 for the two transposes), col 160 ones.
    wm = np.zeros((128, 161), np.float32)
    for k in range(48):
        wm[k, 128 + (k % 16)] = -0.5
        wm[64 + k, 128 + (k % 16)] = -0.5
    for i in range(16):
        wm[i, 144 + i] = 1.0
        wm[32 + i, 144 + i] = 1.0
    wm[:, 160] = 1.0
    return wm


WM_CONST = _build_wm()


def _emit_body(ctx, tc, d):
    nc = tc.nc
    consts = ctx.enter_context(tc.tile_pool(name="consts", bufs=1))
    psum = ctx.enter_context(tc.tile_pool(name="psum", bufs=2, space="PSUM"))
    scratch = ctx.enter_context(tc.tile_pool(name="scratch", bufs=2))

    # ---------------- ramp: input DMAs on 3 queues ----------------
    a1A = consts.tile([6, 512], F32R, name="a1A")
    a1B = consts.tile([6, N - 512], F32R, name="a1B")
    a2 = consts.tile([6, N], F32R, name="a2")
    b1L = consts.tile([6, N // 2], F32R, name="b1L")
    b1R = consts.tile([6, N // 2], F32R, name="b1R")
    b2L = consts.tile([6, N // 2], F32R, name="b2L")
    b2R = consts.tile([6, N // 2], F32R, name="b2R")
    cs1 = consts.tile([3, N], F32R, name="cs1")
    cs2 = consts.tile([3, N], F32R, name="cs2")
    wm = consts.tile([128, 161], F32, name="wm")
    H = N // 2

    # Queue SP: cs2-L, a1-L, cb2-L(b2 rows 3-5), then pass-2 feeds
    nc.sync.dma_start(out=cs2[:, 0:H], in_=d["cs2"][:, 0:H])
    nc.sync.dma_start(out=a1A, in_=d["xa1"][:, 0:512])
    nc.sync.dma_start(out=b2L[3:6, :], in_=d["cb2"][:, 0:H])
    nc.sync.dma_start(out=cs1[:, 0:H], in_=d["cs1"][:, 0:H])
    nc.sync.dma_start(out=a2[:, 0:H], in_=d["xa2"][:, 0:H])
    nc.sync.dma_start(out=b1L[3:6, :], in_=d["cb1"][:, 0:H])
    # Queue Act: cs2-R, cb2-R, dummy table preload, then cp duty
    nc.scalar.dma_start(out=cs2[:, H:N], in_=d["cs2"][:, H:N])
    nc.scalar.dma_start(out=b2R[3:6, :], in_=d["cb2"][:, H:N])
    # Queue Pool: wm, a1-late, then pass-2 feeds
    nc.gpsimd.dma_start(out=wm, in_=d["wm"][:])
    nc.gpsimd.dma_start(out=a1B, in_=d["xa1"][:, 512:N])
    nc.gpsimd.dma_start(out=cs1[:, H:N], in_=d["cs1"][:, H:N])
    nc.gpsimd.dma_start(out=a2[:, H:N], in_=d["xa2"][:, H:N])
    nc.gpsimd.dma_start(out=b1R[3:6, :], in_=d["cb1"][:, H:N])

    # act-table preload (sqrt_and_others covers Copy/Sqrt/Square): one tiny
    # Sqrt on a memset constant absorbs the 1.3us table load at t~0,
    # independent of any input DMA.
    dum = consts.tile([1, 1], F32)
    nc.vector.memset(dum, 4.0)
    dum2 = consts.tile([1, 1], F32)
    nc.scalar.activation(out=dum2, in_=dum,
                         func=mybir.ActivationFunctionType.Sqrt)

    # B2 squares rows: sx = x^2/2 via one stt per half (DVE); R first --
    # it gates the first tile's d2b matmuls (mb 2,3 = cols 1024:2048).
    # Explicit dep pins the order against scheduler reordering.
    sttR = nc.vector.scalar_tensor_tensor(
        out=b2R[0:3, :], in0=cs2[:, H:N], scalar=0.5,
        in1=cs2[:, H:N], op0=ALU.mult, op1=ALU.mult)
    sttL = nc.vector.scalar_tensor_tensor(
        out=b2L[0:3, :], in0=cs2[:, 0:H], scalar=0.5,
        in1=cs2[:, 0:H], op0=ALU.mult, op1=ALU.mult)
    add_dep_helper(sttL.ins, sttR.ins, sync=False,
                   reason="R-half squares first (gates d2b mms)")

    # r tiles for the tail: wide mask-matmul + transpose (idle PE/DVE time)
    sq = consts.tile([128, 128], F32, name="sqw")
    nc.vector.tensor_mul(sq, wm[:, 0:128], wm[:, 0:128])
    rrp = psum.tile([48, 128], F32, tag="d2a", name="rrp")
    nc.tensor.matmul(rrp[0:16, :], wm[0:64, 128:144], sq[0:64, :],
                     start=True, stop=True)
    nc.tensor.matmul(rrp[32:48, :], wm[64:128, 128:144], sq[64:128, :],
                     start=True, stop=True, tile_position=(64, 32))
    rrs = consts.tile([48, 128], F32, name="rrs")
    nc.vector.tensor_copy(rrs[0:16, :], rrp[0:16, :])
    nc.vector.tensor_copy(rrs[32:48, :], rrp[32:48, :])
    t1p = psum.tile([128, 16], F32, tag="d2b", name="t1p")
    nc.tensor.transpose(t1p, rrs[0:16, :], wm[0:16, 144:160])
    rt1 = consts.tile([128, NB], F32, name="rt1")
    nc.vector.tensor_scalar_mul(rt1, t1p, -2.0)  # rt1 = r1 in [p, nb] layout
    t2p = psum.tile([128, 16], F32, tag="d2a", name="t2p")
    nc.tensor.transpose(t2p, rrs[32:48, :], wm[32:48, 144:160],
                        tile_position=(32, 0))
    rt2 = consts.tile([128, NB], F32, name="rt2")
    nc.vector.tensor_scalar_mul(rt2, t2p, -2.0)
    rh1 = consts.tile([128, NB], F32, name="rh1")
    nc.vector.tensor_scalar_mul(rh1, t1p, -1.0)  # r1/2 in [p, nb] layout
    rh2 = consts.tile([128, NB], F32, name="rh2")
    nc.vector.tensor_scalar_mul(rh2, t2p, -1.0)

    # ---------------- main: two passes of 16 x [128, 2048] tiles ----------------
    minsP = consts.tile([128, NB], F32)
    minsQ = consts.tile([128, NB], F32)

    def a1slice(nb):
        if nb < 4:
            return a1A[:, 128 * nb: 128 * (nb + 1)]
        return a1B[:, 128 * nb - 512: 128 * (nb + 1) - 512]

    def tile_pass(lhsT_of, bL, bR, mins, nb, rh, split=False):
        d2b = psum.tile([128, H], F32, tag="d2b")
        d2a = psum.tile([128, H], F32, tag="d2a")
        for mb in (2, 3, 0, 1):
            dst = d2b if mb >= 2 else d2a
            half = bR if mb >= 2 else bL
            nc.tensor.matmul(
                dst[:, 512 * (mb % 2): 512 * (mb % 2 + 1)],
                lhsT_of(nb),
                half[:, 512 * (mb % 2): 512 * (mb % 2 + 1)],
                start=True, stop=True)
        if split:
            # pipeline-start variant: drain+scan per 512-col half; the second
            # scan chains the first via its initial column.
            cpA = scratch.tile([128, 512], F32, tag="cp", bufs=4)
            nc.scalar.copy(cpA, d2b[:, 0:512])
            scA = scratch.tile([128, 512], F32, tag="sc", bufs=6)
            nc.vector.tensor_tensor_scan(
                out=scA, data0=d2b[:, 512:1024], data1=cpA, initial=-1.0e30,
                op0=ALU.max, op1=ALU.max)
            cpB = scratch.tile([128, 512], F32, tag="cp", bufs=4)
            nc.scalar.copy(cpB, d2a[:, 0:512])
            scB = scratch.tile([128, 512], F32, tag="sc", bufs=6)
            nc.vector.tensor_tensor_scan(
                out=scB, data0=d2a[:, 512:1024], data1=cpB,
                initial=scA[:, 511:512], op0=ALU.max, op1=ALU.max)
            nc.gpsimd.tensor_scalar_min(mins[:, nb: nb + 1], scB[:, 511:512],
                                        rh[:, nb: nb + 1])
            return
        cp = scratch.tile([128, H], F32, tag="cp", bufs=4)
        nc.scalar.copy(cp, d2b)
        sc = scratch.tile([128, H], F32, tag="sc", bufs=6)
        nc.vector.tensor_tensor_scan(
            out=sc, data0=d2a, data1=cp, initial=-1.0e30,
            op0=ALU.max, op1=ALU.max)
        # last tile of a pass: extract on DVE (in-order with the scan and
        # the following stt -- skips two cross-engine sem hops in the tail)
        eng = nc.vector if nb == NB - 1 else nc.gpsimd
        eng.tensor_scalar_min(mins[:, nb: nb + 1], sc[:, H - 1: H],
                              rh[:, nb: nb + 1])

    # finalize helper: d^2 = r - 2*max(D-tilde), clamp >= 0, then
    # sqrt(d^2/N^2) = d/N with free-axis sum accumulate (scale folded in so
    # no final multiply is needed).
    def finalize(mins, rt, rs):
        d2 = consts.tile([128, NB], F32)
        nc.vector.scalar_tensor_tensor(out=d2, in0=mins, scalar=-2.0,
                                       in1=rt, op0=ALU.mult, op1=ALU.add)
        sq = consts.tile([128, NB], F32)
        nc.scalar.activation(out=sq, in_=d2,
                             func=mybir.ActivationFunctionType.Sqrt,
                             scale=1.0 / (N * N), accum_out=rs)

    for nb in range(NB):
        tile_pass(a1slice, b2L, b2R, minsP, nb, rh1)
        # B1 squares on the Act engine's per-tile slack (Square, no table
        # thrash: same act set as Copy/Sqrt)
        if nb == 7:
            nc.scalar.activation(out=b1L[0:3, :], in_=cs1[:, 0:H],
                                 func=mybir.ActivationFunctionType.Square,
                                 scale=HALF_SQRT2)
        if nb == 13:
            nc.scalar.activation(out=b1R[0:3, :], in_=cs1[:, H:N],
                                 func=mybir.ActivationFunctionType.Square,
                                 scale=HALF_SQRT2)
    # P-side finalize overlaps pass 2 (only the Q chain trails the last scan)
    rsP = consts.tile([128, 1], F32)
    rsQ = consts.tile([128, 1], F32)
    finalize(minsP, rt1, rsP)
    for nb in range(NB):
        tile_pass(lambda nb: a2[:, 128 * nb: 128 * (nb + 1)], b1L, b1R, minsQ, nb, rh2)
    finalize(minsQ, rt2, rsQ)

    # ---------------- tail ----------------
    # loss = sum_p (rsP + rsQ) via two PSUM-accumulated ones-matmuls
    tot = psum.tile([1, 1], F32, tag="d2b")
    nc.tensor.matmul(tot, wm[:, 160:161], rsP, start=True, stop=False,
                     skip_group_check=True)
    nc.tensor.matmul(tot, wm[:, 160:161], rsQ, start=False, stop=True,
                     skip_group_check=True)
    res = consts.tile([1, 1], F32)
    nc.vector.tensor_copy(res, tot[0:1, 0:1])
    nc.sync.dma_start(out=d["out"][:], in_=res)


def build_nc(cap_waits=True):
    nc = bass.Bass()
    d = {
        "xa1": nc.declare_dram_parameter("xa1", [6, N], F32R, isOutput=False),
        "xa2": nc.declare_dram_parameter("xa2", [6, N], F32R, isOutput=False),
        "cb1": nc.declare_dram_parameter("cb1", [3, N], F32R, isOutput=False),
        "cb2": nc.declare_dram_parameter("cb2", [3, N], F32R, isOutput=False),
        "cs1": nc.declare_dram_parameter("cs1", [3, N], F32R, isOutput=False),
        "cs2": nc.declare_dram_parameter("cs2", [3, N], F32R, isOutput=False),
        "wm": nc.declare_dram_parameter("wm", [128, 161], F32, isOutput=False),
        "out": nc.declare_dram_parameter("out", [1, 1], F32, isOutput=True),
    }
    with tile.TileContext(nc) as tc, ExitStack() as ctx:
        _emit_body(ctx, tc, d)
    if cap_waits:
        _cap_sync_waits(nc)
    return nc


_CACHE = {}


def make_in_maps(set1, set2):
    set1 = np.asarray(set1, dtype=np.float32)
    set2 = np.asarray(set2, dtype=np.float32)
    in_maps = []
    for b in range(B):
        c1 = np.ascontiguousarray(set1[b].T)  # [3, N]
        c2 = np.ascontiguousarray(set2[b].T)
        xa1 = np.concatenate([np.full((3, N), -1.0, np.float32), c1], axis=0)
        xa2 = np.concatenate([np.full((3, N), -1.0, np.float32), c2], axis=0)
        wm = WM_CONST.copy()
        z = np.zeros((1, N), np.float32)
        xt = np.concatenate([c1, z, c2, z], axis=0)  # [8, N]
        wm[:, 0:128] = xt.reshape(8, 16, 128).reshape(128, 128)
        in_maps.append({"xa1": xa1, "xa2": xa2, "cb1": c1, "cb2": c2,
                        "cs1": c1, "cs2": c2, "wm": wm})
    return in_maps


def kernel(set1, set2, _trace=False):
    if "nc" not in _CACHE:
        _CACHE["nc"] = build_nc()
    nc = _CACHE["nc"]
    r = run_bass_kernel_spmd(nc, make_in_maps(set1, set2),
                             core_ids=list(range(B)), trace=_trace)
    _CACHE["last_result"] = r
    return np.array([r.results[b]["out"][0, 0] for b in range(B)],
                    dtype=np.float32)
